# revision 57
# baseline (speedup 1.0000x reference)
"""Trainium2 Bass kernel for nn_LocalAttentionParallel.

Reference computation (B=4, T=4096, D=768, span=256):
    q/k/v = Linear(gelu(Linear(x)))   (three 768->768->768 MLPs, exact gelu)
    scores = (q @ k^T) / sqrt(D*span), banded causal mask (0 <= i-j < span), NO softmax
    y = scores @ v ; out = layernorm(y) * ln_w + ln_b

Key algebraic optimizations (no softmax => scores are bilinear):
    scores_ij = q_i . k_j  with  k_j = g2_j @ kw2 + kb2,  q_i = g1_i @ qw2 + qb2
              = g2_j . q''_i + alpha_i
    where (host-precomputed folds)
        Mqk = qw2 @ kw2^T,  b'' = qb2 @ kw2^T,  u = qw2 @ kb2,  c = qb2 . kb2
        q'' = g1 @ Mqk + b''            (replaces the q stage-2 GEMM)
        alpha = g1 @ u + c              (M=1 GEMV + K=1 rank-1 matmul into scores)
    so the k-side stage-2 GEMM (2304 rows/core incl. halo) is eliminated.

    LN mean fold: mean_d(y_i) = sum_j S_ij * mean_d(v_j), so centering the
    ROWS of v makes y exactly zero-mean. v rows are centered for free by
    centering vw2's rows and vb2 on the host:
        vw2_c[k,:] = vw2[k,:] - mean(vw2[k,:]),  vb2_c = vb2 - mean(vb2)
    Then LN reduces to y * rsqrt(var+eps); the scale runs split across
    DVE (cols 0:384) and ACT (Copy with per-partition scale, cols 384:768).

Sharding: 8 cores = batch(4) x sequence-halves(2). Each core processes 2048
own rows plus a 256-row left halo (zeros for the first half; handled by
per-core boundary masks). All sharding/layout prep happens on the host; the
device kernel is SPMD-uniform.

Head/DMA: all large tensors ride ONE HWDGE queue (SP/sync) in strict
first-need order (xp, kw1(2 halves), xb0, xb1, qw1(2 halves), vw1, vw2,
mqk) so arrival order matches the compute schedule -- exactly 11 critical
transfers, fitting the ~11-deep DMA completion-semaphore pool so no issue
ever blocks. The compute prologue is WEIGHT-major (all kw1 stage-1 work for
halo+blocks 0-1, then qw1's, then vw1's) so each 1.2MB weight buys ~19us of
PE work (~0.14 MB/us need-rate vs the queue's ~0.2+ MB/us delivery); the
main loop prefetches stage-1 of block b+2 before stage-2/attention of
block b, giving the tail weights (vw2, mqk) ~40us of runway. Engine split:
PE matmuls; ACT gelus + q'' bias-adds (Identity, co-resident with Gelu in
the table set -- no swap); DVE score masking + LN stats/rsqrt/scale;
Pool/SWDGE small-constant DMA + early-block y stores.
"""

import os
import numpy as np

import concourse.bass as bass
import concourse.tile as tile
import concourse.mybir as mybir
from concourse import bacc
from concourse.bass_utils import run_bass_kernel_spmd

AF = mybir.ActivationFunctionType
ALU = mybir.AluOpType

# problem constants
B, T, D = 4, 4096, 768
SPAN = 256
LN_EPS = 1e-5
SCALE = float(np.sqrt(D * SPAN))

P = 128
NCH = D // P          # 6 contraction chunks
N_CORES = 8
T_OWN = T // 2        # rows owned per core (2048)
T_LOC = T_OWN + SPAN  # rows incl. halo (2304)
TQ = T_OWN // P       # 16 query tiles
RING = 5
N_WARM = 13

# compute dtype: "f16" | "f32r" | "bf16" | "f32"
CDT_NAME = os.environ.get("TRN_KERNEL_CDT", "f16")
_DT = {
    "f16": (mybir.dt.float16, np.float16),
    "bf16": (mybir.dt.bfloat16, None),  # ml_dtypes.bfloat16 resolved lazily
    "f32r": (mybir.dt.float32r, np.float32),
    "f32": (mybir.dt.float32, np.float32),
}
TPB = 4               # max tiles per block (512 moving columns)
BC = TPB * P
# (start_tile, n_tiles): tapered tail so the final LN/store chain after the
# last matmul is short
BLOCKS = [(0, 4), (4, 4), (8, 4), (12, 3), (15, 1)]
assert sum(n for _, n in BLOCKS) == TQ
F32 = mybir.dt.float32
I32 = mybir.dt.int32
F16 = mybir.dt.float16


def _np_cdt():
    if CDT_NAME == "bf16":
        import ml_dtypes
        return ml_dtypes.bfloat16
    return _DT[CDT_NAME][1]


def build_module(apply_ln: bool, c_alpha: float):
    cdt = _DT[CDT_NAME][0]
    nc = bacc.Bacc("TRN2", target_bir_lowering=False, debug=False,
                   num_devices=N_CORES)

    xT = nc.dram_tensor("xT", [P, NCH * T_LOC], cdt, kind="ExternalInput")
    wd = {}
    # kw1 is stored third-major and qw1 half-major so each m-slice is one
    # contiguous DMA (a strided slice of [P, c, D] would shatter into 768B
    # packets and crawl); kw1's first third lands ~1us earlier than a half
    # would, starting the pipeline sooner. vw2+mqk ride as ONE merged
    # transfer (they arrive last back-to-back) keeping the critical
    # transfer count at exactly 11 = the DMA completion-semaphore pool.
    wd["kw1"] = nc.dram_tensor("kw1", [P, 3, NCH, 2 * P], cdt,
                               kind="ExternalInput")
    wd["qw1"] = nc.dram_tensor("qw1", [P, 2, NCH, 3 * P], cdt,
                               kind="ExternalInput")
    wd["s2w"] = nc.dram_tensor("s2w", [P, 2, NCH, D], cdt,
                               kind="ExternalInput")
    # vw1 is m-major so stage1-v's m-loop walks contiguous memory
    vw1d = nc.dram_tensor("vw1", [P, NCH, NCH, P], cdt, kind="ExternalInput")
    ud = nc.dram_tensor("u", [P, NCH], cdt, kind="ExternalInput")
    b1d = nc.dram_tensor("b1", [P, 3 * NCH], F32, kind="ExternalInput")
    b2d = nc.dram_tensor("b2", [P, NCH], F32, kind="ExternalInput")
    vb2d = nc.dram_tensor("vb2bc", [P, D], F32, kind="ExternalInput")
    maskd = nc.dram_tensor("masks", [P, 2, 3 * P], F32, kind="ExternalInput")
    if apply_ln:
        lnwd = nc.dram_tensor("lnw", [P, D], F32, kind="ExternalInput")
        lnbd = nc.dram_tensor("lnb", [P, D], F32, kind="ExternalInput")
    yd = nc.dram_tensor("y", [P, TQ, D], F16, kind="ExternalOutput")

    with tile.TileContext(nc) as tc:
        with (
            tc.tile_pool(name="work", bufs=1) as wp,
            tc.tile_pool(name="psum", bufs=1, space="PSUM") as pp,
        ):
            cp = wp  # single SBUF pool: one fewer pool-exit barrier round
            # ---- small constants (no DMA) ----
            onest = cp.tile([P, P], cdt, tag="onest", name="onest")
            nc.vector.memset(onest, 1.0)

            # ---- PE warm-up input: dummy matmuls release the HAM clock gate
            # while the first weight DMAs are still in flight ----
            wmt = wp.tile([P, 256], cdt, tag="warm", bufs=1, name="warm")
            nc.vector.memset(wmt, 0.0)

            # Warm-up train: keeps the PE busy from ~7.6us (end of framework
            # preamble) until the first weights land (~11-13us), so HAM
            # un-throttles at ~11us and the PE never re-throttles.
            for _ in range(N_WARM):
                wps = pp.tile([P, 512], F32, tag="psA", bufs=2, name="psA")
                nc.tensor.matmul(wps[:, :256], wmt[:, :P], wmt[:, :256],
                                 start=True, stop=True)

            # ---- input DMA ----
            # SWDGE (gpsimd) carries the small constants; b1 first (needed by
            # the first gelu). ALL big tensors ride the single SP/sync HWDGE
            # queue in strict first-need order, so a single ~0.36 GB/us HBM
            # stream delivers each tensor exactly when the pipeline reaches
            # it. (Two parallel queues would halve each stream's rate and
            # delay the FIRST weights, which gate the pipeline start.)
            b1t = cp.tile([P, 3 * NCH], F32, tag="b1t", name="b1t")
            nc.gpsimd.dma_start(out=b1t, in_=b1d[:])

            xp = wp.tile([P, NCH, SPAN], cdt, tag="xpp", bufs=1, name="xpp")
            nc.sync.dma_start(out=xp[:].rearrange("p c n -> p (c n)"),
                              in_=xT[:, 0:NCH * SPAN])
            wsb = {}
            wsb["kw1"] = cp.tile([P, 3, NCH, 2 * P], cdt, tag="w_kw1",
                                 name="w_kw1")
            wsb["qw1"] = cp.tile([P, 2, NCH, 3 * P], cdt, tag="w_qw1",
                                 name="w_qw1")
            s2wt = cp.tile([P, 2, NCH, D], cdt, tag="w_s2w", name="w_s2w")
            wsb["vw2"] = s2wt[:, 0]
            wsb["mqk"] = s2wt[:, 1]
            vw1t = cp.tile([P, NCH, NCH, P], cdt, tag="w_vw1", name="w_vw1")

            def xb_tile():
                return wp.tile([P, NCH, BC], cdt, tag="xT", bufs=3,
                               name="xTb")

            def xb_dma(xb, s0, ncols):
                off = NCH * (SPAN + s0 * P)
                if ncols == BC:
                    dst = xb[:].rearrange("p c n -> p (c n)")
                else:
                    dst = xb[:, :, :ncols]
                nc.sync.dma_start(out=dst, in_=xT[:, off:off + NCH * ncols])

            # ALL big tensors ride the single SP/sync queue in strict
            # first-need order (measured: one queue sustains ~0.2 MB/us in
            # the head; a second concurrent queue just splits that rate and
            # delays the first weights, which gate the pipeline start).
            # kw1/qw1 are halved so the first half lands ~3us earlier. The
            # compute prologue is WEIGHT-major (all kw1 work for halo+b0+b1,
            # then all qw1 work, then vw1) so each 1.2MB weight buys ~19us
            # of PE work -- need rate ~0.14 MB/us < DMA's ~0.2.
            def w_half_dma(nm, hh):
                nc.sync.dma_start(
                    out=wsb[nm][:, hh].rearrange("p c n -> p (c n)"),
                    in_=wd[nm][:, hh].rearrange("p c n -> p (c n)"))

            for tt in (0, 1, 2):
                nc.sync.dma_start(
                    out=wsb["kw1"][:, tt].rearrange("p c n -> p (c n)"),
                    in_=wd["kw1"][:, tt].rearrange("p c n -> p (c n)"))
            xb0 = xb_tile()
            xb_dma(xb0, 0, BC)
            xb1 = xb_tile()
            xb_dma(xb1, BLOCKS[1][0], BLOCKS[1][1] * P)
            w_half_dma("qw1", 0)
            w_half_dma("qw1", 1)
            for mlo in (0, 3):
                nc.sync.dma_start(
                    out=vw1t[:, mlo:mlo + 3].rearrange("p a c n -> p (a c n)"),
                    in_=vw1d[:, mlo:mlo + 3].rearrange("p a c n -> p (a c n)"))
            nc.sync.dma_start(
                out=s2wt[:].rearrange("p a c n -> p (a c n)"),
                in_=wd["s2w"][:].rearrange("p a c n -> p (a c n)"))

            # Small constants ride the TAIL of the same sync issue list: the
            # 11 critical transfers above fit exactly in the ~11-deep DMA
            # completion-semaphore pool, so none of them ever blocks at
            # issue; these four block briefly (needed only at ~50us+).
            vb2t = cp.tile([P, D], F32, tag="vb2t", name="vb2t")
            nc.sync.dma_start(out=vb2t, in_=vb2d[:])
            maskt = cp.tile([P, 2, 3 * P], F32, tag="maskt", name="maskt")
            nc.sync.dma_start(out=maskt, in_=maskd[:])
            ut = cp.tile([P, NCH], cdt, tag="ut", name="ut")
            nc.sync.dma_start(out=ut, in_=ud[:])
            b2t = cp.tile([P, NCH], F32, tag="b2t", name="b2t")
            nc.sync.dma_start(out=b2t, in_=b2d[:])
            if apply_ln:
                lnwt = cp.tile([P, D], F32, tag="lnwt", name="lnwt")
                nc.gpsimd.dma_start(out=lnwt, in_=lnwd[:])
                lnbt = cp.tile([P, D], F32, tag="lnbt", name="lnbt")
                nc.gpsimd.dma_start(out=lnbt, in_=lnbd[:])

            def ps512(ncols):
                t = pp.tile([P, 512], F32, tag="psA", bufs=2, name="psA")
                return t[:, :ncols]

            def ps768():
                return pp.tile([P, D], F32, tag="psB", bufs=3, name="psB")

            def w1_lhsT(w1, m, c):
                if w1 == "vw1":
                    return vw1t[:, m, c, :]
                if w1 == "kw1":
                    return wsb[w1][:, m // 2, c,
                                   (m % 2) * P:(m % 2 + 1) * P]
                return wsb[w1][:, m // 3, c, (m % 3) * P:(m % 3 + 1) * P]

            def stage1(xblk, w1, bj, ncols, out_tile=None, tag=None):
                """h = gelu(w1.T @ xT + b1) -> [P, NCH, ncols] (cdt)."""
                h = out_tile
                if h is None:
                    h = wp.tile([P, NCH, BC], cdt, tag=tag, bufs=3, name=tag)
                for m in range(NCH):
                    ps = ps512(ncols)
                    for c in range(NCH):
                        nc.tensor.matmul(
                            ps, w1_lhsT(w1, m, c),
                            xblk[:, c, :ncols],
                            start=(c == 0), stop=(c == NCH - 1))
                    nc.scalar.activation(h[:, m, :ncols], ps, AF.Gelu,
                                         bias=b1t[:, bj * NCH + m:bj * NCH + m + 1],
                                         scale=1.0)
                return h

            def stage2_q(h, qT, ncols):
                """q'' = Mqk.T @ h + b''; also alpha = u.T @ h + c.
                The alpha GEMV runs FIRST so its DVE consumer (asb add)
                drains during the six o-group matmuls -- otherwise the
                second scores group stalls on the alpha psA slot."""
                aps = pp.tile([P, 512], F32, tag="psA", bufs=2, name="psA")
                for m in range(NCH):
                    nc.tensor.matmul(
                        aps[0:1, :ncols], ut[:, m:m + 1], h[:, m, :ncols],
                        start=(m == 0), stop=(m == NCH - 1))
                asb = wp.tile([P, BC], cdt, tag="alph", bufs=2, name="alph")
                nc.vector.tensor_scalar_add(
                    asb[0:1, :ncols], aps[0:1, :ncols], c_alpha)
                for o in range(NCH):
                    ps = ps512(ncols)
                    for m in range(NCH):
                        nc.tensor.matmul(
                            ps, wsb["mqk"][:, m, o * P:(o + 1) * P],
                            h[:, m, :ncols],
                            start=(m == 0), stop=(m == NCH - 1))
                    # Identity is co-resident with Gelu in the ACT table
                    # set, so this runs on ACT with no table swap -- keeps
                    # score-prep off DVE's LN-burst FIFO
                    nc.scalar.activation(qT[:, o, :ncols], ps, AF.Identity,
                                         bias=b2t[:, o:o + 1], scale=1.0)
                return asb

            def stage2_v(h, vslot, t0, ntiles):
                """v row-tiles [rows, D] = h.T @ vw2_c + vb2_c (row-centered
                by the host-side weight fold)."""
                for t in range(ntiles):
                    ps = ps768()
                    for c0, cw in ((0, 512), (512, 256)):
                        for m in range(NCH):
                            nc.tensor.matmul(
                                ps[:, c0:c0 + cw],
                                h[:, m, t * P:(t + 1) * P],
                                wsb["vw2"][:, m, c0:c0 + cw],
                                start=(m == 0), stop=(m == NCH - 1))
                    nc.vector.tensor_add(vslot[:, t0 + t, :], ps, vb2t)

            def new_kv_slot():
                k = wp.tile([P, NCH, BC], cdt, tag="kring", bufs=RING,
                            name="kring")
                v = wp.tile([P, TPB, D], cdt, tag="vring", bufs=RING,
                            name="vring")
                return k, v

            # k/v tiles tracked by absolute tile index: kt -> (tile, pos)
            ktile = {}
            vtile = {}
            kvslot = {}

            def stage1_block(b):
                """stage-1 (q,k,v) for block b; returns (hq, hv)."""
                s0, n = BLOCKS[b]
                ncols = n * P
                if b == 0:
                    xb = xb0
                elif b == 1:
                    xb = xb1
                else:
                    xb = xb_tile()
                    xb_dma(xb, s0, ncols)
                hq = stage1(xb, "qw1", 0, ncols, tag="hq")
                kb, vb = new_kv_slot()
                kvslot[b] = (kb, vb)
                for t in range(n):
                    ktile[s0 + 2 + t] = (kb, t)
                    vtile[s0 + 2 + t] = (vb, t)
                stage1(xb, "kw1", 1, ncols, out_tile=kb)
                hv = stage1(xb, "vw1", 2, ncols, tag="hv")
                return hq, hv

            # ---- prologue: weight-major stage-1 over halo + blocks 0,1
            # (k stage-2 eliminated: the scores use g2 directly). All kw1
            # work first, then all qw1, then all vw1 -- matching the
            # single-queue DMA arrival order so the PE never waits long.
            kp, vp = new_kv_slot()
            ktile[0] = (kp, 0)
            ktile[1] = (kp, 1)
            vtile[0] = (vp, 0)
            vtile[1] = (vp, 1)
            s1, n1 = BLOCKS[1]
            kb0, vb0 = new_kv_slot()
            kvslot[0] = (kb0, vb0)
            for t in range(BLOCKS[0][1]):
                ktile[2 + t] = (kb0, t)
                vtile[2 + t] = (vb0, t)
            kb1, vb1 = new_kv_slot()
            kvslot[1] = (kb1, vb1)
            for t in range(n1):
                ktile[s1 + 2 + t] = (kb1, t)
                vtile[s1 + 2 + t] = (vb1, t)
            st1 = {}
            stage1(xp, "kw1", 1, SPAN, out_tile=kp)
            stage1(xb0, "kw1", 1, BC, out_tile=kb0)
            stage1(xb1, "kw1", 1, n1 * P, out_tile=kb1)
            hq0 = stage1(xb0, "qw1", 0, BC, tag="hq")
            hq1 = stage1(xb1, "qw1", 0, n1 * P, tag="hq")
            hv_halo = stage1(xp, "vw1", 2, SPAN, tag="hv")
            hv0 = stage1(xb0, "vw1", 2, BC, tag="hv")
            hv1 = stage1(xb1, "vw1", 2, n1 * P, tag="hv")
            st1[0] = (hq0, hv0)
            st1[1] = (hq1, hv1)
            # halo v stage-2 (first vw2 use; vw2 has landed long before)
            stage2_v(hv_halo, vp, 0, 2)

            def stage2_block(bb):
                hq, hv = st1.pop(bb)
                stage2_v(hv, kvslot[bb][1], 0, BLOCKS[bb][1])
                qT = wp.tile([P, NCH, BC], cdt, tag="qT", bufs=2, name="qT")
                asb = stage2_q(hq, qT, BLOCKS[bb][1] * P)
                return qT, asb

            s2 = {}

            # ---- main loop over blocks: prefetch stage-1 of block b+2,
            # then stage-2 + attention of block b. The FINAL block's stage-2
            # is hoisted before the previous block's attention so its
            # score-prep (DVE alpha, ACT qT adds) is enqueued ahead of the
            # taper's LN bursts and never stalls the PE ----
            for b, (s0, n) in enumerate(BLOCKS):
                ncols = n * P
                if b + 2 < len(BLOCKS):
                    st1[b + 2] = stage1_block(b + 2)
                s2[b] = stage2_block(b)
                qT, asb = s2.pop(b)

                # ---- attention: scoresT per k-tile over the whole q block.
                # Groups are emitted WIDEST-W first: the wide groups'
                # ~160ns/chunk cadence hides the final qT o-group add
                # (ACT, ~690ns) that the c=5 chunks depend on; narrow
                # W=128 groups (56ns/chunk) cannot. ----
                scs = {}
                dlts = sorted(range(n + 2),
                              key=lambda d: -(min(n - 1, d) - max(0, d - 2)))
                for dlt in dlts:
                    p_lo = max(0, dlt - 2)
                    p_hi = min(n - 1, dlt)
                    W = (p_hi - p_lo + 1) * P
                    qoff = p_lo * P
                    moff = (2 - (dlt - p_lo)) * P
                    msel = 1 if (b == 0 and dlt < 2) else 0
                    kts, kpos = ktile[s0 + dlt]
                    ps = ps512(W)
                    for c in range(NCH):
                        nc.tensor.matmul(
                            ps, kts[:, c, kpos * P:(kpos + 1) * P],
                            qT[:, c, qoff:qoff + W],
                            start=(c == 0), stop=False)
                    # + alpha_i broadcast over keys (rank-1, K=1)
                    nc.tensor.matmul(
                        ps, onest[0:1, :P], asb[0:1, qoff:qoff + W],
                        start=False, stop=True)
                    sc = wp.tile([P, 3 * P], cdt, tag="sc", bufs=8, name="sc")
                    nc.vector.tensor_mul(sc[:, :W], ps,
                                         maskt[:, msel, moff:moff + W])
                    scs[dlt] = (sc, qoff)

                # y = scoresT.T @ v accumulated over the 3 band tiles,
                # chunk-major so bn_stats on cols [0:512] overlaps the
                # [512:768] matmuls. v rows are pre-centered, so y is
                # zero-mean and LN is y * rsqrt(var+eps): rsqrt via DVE
                # magic-seed + 1 Newton step (batched over tile pairs),
                # scale split DVE (cols 0:384) / ACT Copy (cols 384:768).
                yb = wp.tile([P, TPB, D], F16, tag="yb", bufs=3, name="yb")
                mvb = wp.tile([P, TPB, 2], F32, tag="mvb", bufs=2, name="mvb")
                for p in range(n):
                    if b == len(BLOCKS) - 1 and not apply_ln:
                        # final block's single y tile accumulates in TWO psA
                        # slots (512+256) instead of psB, so it never waits
                        # on block 3's LN chains to free a psB slot
                        psy0 = ps512(512)
                        psy1 = ps512(256)
                        chunks = ((psy0, 0, 512), (psy1, 512, 256))
                    else:
                        psy = ps768()
                        chunks = ((psy, 0, 512), (psy, 512, 256))
                    st = wp.tile([P, 2, 6], F32, tag="st", bufs=4, name="st")
                    for si, (pst, c0v, cw) in enumerate(chunks):
                        dst = pst[:, 0:cw] if pst is not chunks[0][0] \
                            else pst[:, c0v:c0v + cw]
                        for j, dlt in enumerate((p, p + 1, p + 2)):
                            sc, qoff = scs[dlt]
                            soff = p * P - qoff
                            vts, vpos = vtile[s0 + dlt]
                            nc.tensor.matmul(
                                dst,
                                sc[:, soff:soff + P],
                                vts[:, vpos, c0v:c0v + cw],
                                start=(j == 0), stop=(j == 2))
                        # chunk-aligned stats groups: group 0 (512 cols)
                        # fully overlaps the second chunk's MMs
                        nc.vector.bn_stats(st[:, si, :], dst)
                    nc.vector.bn_aggr(mvb[:, p, :], st)
                    # rr = rsqrt(var + eps)
                    vh = wp.tile([P, 1], F32, tag="vh", bufs=2, name="vh")
                    rr = wp.tile([P, 1], F32, tag="rr", bufs=2, name="rr")
                    ta = wp.tile([P, 1], F32, tag="ta", bufs=2, name="ta")
                    v2 = wp.tile([P, 1], F32, tag="v2", bufs=2, name="v2")
                    nc.vector.tensor_scalar_add(v2, mvb[:, p, 1:2], LN_EPS)
                    nc.vector.tensor_scalar_mul(vh, v2, -0.5)
                    nc.vector.tensor_copy(ta, v2[:].bitcast(I32))
                    nc.vector.tensor_scalar(
                        out=ta, in0=ta, scalar1=-0.5,
                        scalar2=float(0x5F375A86), op0=ALU.mult, op1=ALU.add)
                    nc.vector.tensor_copy(rr[:].bitcast(I32), ta)
                    nc.vector.tensor_mul(ta, rr, rr)
                    nc.vector.tensor_scalar(
                        out=ta, in0=ta, scalar1=vh[:, 0:1], scalar2=1.5,
                        op0=ALU.mult, op1=ALU.add)
                    nc.vector.tensor_mul(rr, rr, ta)
                    if apply_ln:
                        yf = wp.tile([P, D], F32, tag="yf", bufs=2, name="yf")
                        nc.vector.tensor_scalar(
                            out=yf, in0=psy,
                            scalar1=mvb[:, p, 0:1], scalar2=rr[:, 0:1],
                            op0=ALU.subtract, op1=ALU.mult)
                        nc.vector.tensor_mul(yf, yf, lnwt)
                        nc.vector.tensor_add(yb[:, p, :], yf, lnbt)
                    elif b == len(BLOCKS) - 1:
                        # final block: no gelus follow, so the ACT engine is
                        # free -- split the scale DVE/ACT and the store in
                        # halves to shorten the post-last-matmul chain
                        nc.vector.tensor_scalar_mul(yb[:, p, 0:384],
                                                    psy0[:, 0:384],
                                                    rr[:, 0:1])
                        nc.scalar.activation(
                            yb[:, p, 384:512], psy0[:, 384:512], AF.Copy,
                            bias=0.0, scale=rr[:, 0:1])
                        nc.scalar.activation(
                            yb[:, p, 512:768], psy1[:, 0:256], AF.Copy,
                            bias=0.0, scale=rr[:, 0:1])
                        # NOTE: issuing this store on gpsimd instead of sync
                        # triggers a global ~20ns/MM slowdown (+46us!) --
                        # keep it on the sync ring
                        nc.sync.dma_start(out=yd[:, s0 + p, :],
                                          in_=yb[:, p, :])
                    else:
                        # v rows are host-centered, so y is zero-mean:
                        # normalize is a pure per-row scale by rr
                        nc.vector.tensor_scalar_mul(yb[:, p, :], psy,
                                                    rr[:, 0:1])
                    if b < len(BLOCKS) - 1:
                        yeng = nc.sync if b >= len(BLOCKS) - 2 else nc.gpsimd
                        yeng.dma_start(out=yd[:, s0 + p, :], in_=yb[:, p, :])

    nc.compile()
    return nc


def _make_masks(h):
    jj, ii = np.mgrid[0:P, 0:P]
    diag = (ii >= jj).astype(np.float32)
    full = np.ones((P, P), np.float32)
    strict = (ii < jj).astype(np.float32)
    gen = np.concatenate([diag, full, strict], axis=1) / SCALE
    if h == 0:
        z = np.zeros((P, P), np.float32)
        blk0 = np.concatenate([diag / SCALE, z, z], axis=1)
    else:
        blk0 = gen
    return np.stack([gen, blk0], axis=1)  # [P, 2, 384]


def kernel(**inputs):
    x = np.asarray(inputs["x"], np.float32)
    npdt = _np_cdt()

    ln_w = np.asarray(inputs["ln_w"], np.float32)
    ln_b = np.asarray(inputs["ln_b"], np.float32)
    apply_ln = not (np.all(ln_w == 1.0) and np.all(ln_b == 0.0))

    qw2 = np.asarray(inputs["qw2"], np.float32)
    kw2 = np.asarray(inputs["kw2"], np.float32)
    qb2 = np.asarray(inputs["qb2"], np.float32)
    kb2 = np.asarray(inputs["kb2"], np.float32)
    mqk = qw2 @ kw2.T                      # [768, 768]
    bpp = qb2 @ kw2.T                      # [768]
    u = qw2 @ kb2                          # [768]
    c_alpha = float(qb2 @ kb2)

    nc = build_module(apply_ln, c_alpha)

    def warr(w):
        w = np.asarray(w, np.float32)
        return np.ascontiguousarray(
            w.reshape(NCH, P, D).transpose(1, 0, 2)).astype(npdt)

    def warr_halves(w):
        # [P, c, D] -> [P, half, c, 3*P] (m-halves contiguous for DMA)
        a = warr(w).reshape(P, NCH, 2, 3 * P)
        return np.ascontiguousarray(a.transpose(0, 2, 1, 3))

    def warr_thirds(w):
        # [P, c, D] -> [P, third, c, 2*P] (m-thirds contiguous for DMA)
        a = warr(w).reshape(P, NCH, 3, 2 * P)
        return np.ascontiguousarray(a.transpose(0, 2, 1, 3))

    # center v rows via the weights: vw2_c rows sum to zero, vb2_c mean 0
    vw2 = np.asarray(inputs["vw2"], np.float32)
    vw2_c = vw2 - vw2.mean(axis=1, keepdims=True)
    vb2 = np.asarray(inputs["vb2"], np.float32)
    vb2_c = vb2 - vb2.mean()

    # vw1 m-major: vw1m[p, m, c, col] = vw1[c*P+p, m*P+col]
    vw1m = np.ascontiguousarray(
        np.asarray(inputs["vw1"], np.float32)
        .reshape(NCH, P, NCH, P).transpose(1, 2, 0, 3)).astype(npdt)
    wmats = {
        "qw1": warr_halves(inputs["qw1"]), "kw1": warr_thirds(inputs["kw1"]),
        "vw1": vw1m,
        "s2w": np.ascontiguousarray(
            np.stack([warr(vw2_c), warr(mqk)], axis=1)),
    }
    b1 = np.ascontiguousarray(
        np.stack([inputs["qb1"], inputs["kb1"], inputs["vb1"]])
        .astype(np.float32).reshape(3, NCH, P).transpose(2, 0, 1)
        .reshape(P, 3 * NCH))
    b2 = np.ascontiguousarray(bpp.reshape(NCH, P).transpose(1, 0))
    u_arr = np.ascontiguousarray(
        u.reshape(NCH, P).transpose(1, 0)).astype(npdt)
    vb2bc = np.ascontiguousarray(np.broadcast_to(vb2_c, (P, D)))

    in_maps = []
    for core in range(N_CORES):
        bi, h = core // 2, core % 2
        xl = np.zeros((T_LOC, D), np.float32)
        lo = h * T_OWN - SPAN
        if h == 0:
            xl[SPAN:] = x[bi, 0:T_OWN]
        else:
            xl[:] = x[bi, lo:lo + T_LOC]
        xt3 = xl.T.reshape(NCH, P, T_LOC).transpose(1, 0, 2)  # [P, c, r]
        segs = [xt3[:, :, 0:SPAN].reshape(P, -1)]
        for (s0, n) in BLOCKS:
            c0 = SPAN + s0 * P
            segs.append(xt3[:, :, c0:c0 + n * P].reshape(P, -1))
        xTn = np.ascontiguousarray(np.concatenate(segs, axis=1)).astype(npdt)
        m = {
            "xT": xTn, "b1": b1, "b2": b2, "u": u_arr, "vb2bc": vb2bc,
            "masks": np.ascontiguousarray(_make_masks(h)),
        }
        m.update(wmats)
        if apply_ln:
            m["lnw"] = np.ascontiguousarray(np.broadcast_to(ln_w, (P, D)))
            m["lnb"] = np.ascontiguousarray(np.broadcast_to(ln_b, (P, D)))
        in_maps.append(m)

    trace = os.environ.get("TRN_KERNEL_TRACE", "0") == "1"
    res = run_bass_kernel_spmd(nc, in_maps, core_ids=list(range(N_CORES)),
                               trace=trace)
    if trace and res.exec_time_ns is not None:
        print(f"HW exec time: {res.exec_time_ns} ns")
        print(f"mean exec time: {res.mean_exec_time_ns} ns")
        if res.instructions_and_trace is not None:
            print(f"trace: {res.instructions_and_trace[1]}")

    out = np.empty((B, T, D), np.float32)
    for core in range(N_CORES):
        bi, h = core // 2, core % 2
        yc = np.asarray(res.results[core]["y"], np.float32)  # [P, TQ, D]
        out[bi, h * T_OWN:(h + 1) * T_OWN] = (
            yc.transpose(1, 0, 2).reshape(T_OWN, D))
    return out


# revision 58
# speedup vs baseline: 1.0039x; 1.0039x over previous
"""Trainium2 Bass kernel for nn_LocalAttentionParallel.

Reference computation (B=4, T=4096, D=768, span=256):
    q/k/v = Linear(gelu(Linear(x)))   (three 768->768->768 MLPs, exact gelu)
    scores = (q @ k^T) / sqrt(D*span), banded causal mask (0 <= i-j < span), NO softmax
    y = scores @ v ; out = layernorm(y) * ln_w + ln_b

Key algebraic optimizations (no softmax => scores are bilinear):
    scores_ij = q_i . k_j  with  k_j = g2_j @ kw2 + kb2,  q_i = g1_i @ qw2 + qb2
              = g2_j . q''_i + alpha_i
    where (host-precomputed folds)
        Mqk = qw2 @ kw2^T,  b'' = qb2 @ kw2^T,  u = qw2 @ kb2,  c = qb2 . kb2
        q'' = g1 @ Mqk + b''            (replaces the q stage-2 GEMM)
        alpha = g1 @ u + c              (M=1 GEMV + K=1 rank-1 matmul into scores)
    so the k-side stage-2 GEMM (2304 rows/core incl. halo) is eliminated.

    LN mean fold: mean_d(y_i) = sum_j S_ij * mean_d(v_j), so centering the
    ROWS of v makes y exactly zero-mean. v rows are centered for free by
    centering vw2's rows and vb2 on the host:
        vw2_c[k,:] = vw2[k,:] - mean(vw2[k,:]),  vb2_c = vb2 - mean(vb2)
    Then LN reduces to y * rsqrt(var+eps); the scale runs split across
    DVE (cols 0:384) and ACT (Copy with per-partition scale, cols 384:768).

Sharding: 8 cores = batch(4) x sequence-halves(2). Each core processes 2048
own rows plus a 256-row left halo (zeros for the first half; handled by
per-core boundary masks). All sharding/layout prep happens on the host; the
device kernel is SPMD-uniform.

Head/DMA: all large tensors ride ONE HWDGE queue (SP/sync) in strict
first-need order (xp, kw1(2 halves), xb0, xb1, qw1(2 halves), vw1, vw2,
mqk) so arrival order matches the compute schedule -- exactly 11 critical
transfers, fitting the ~11-deep DMA completion-semaphore pool so no issue
ever blocks. The compute prologue is WEIGHT-major (all kw1 stage-1 work for
halo+blocks 0-1, then qw1's, then vw1's) so each 1.2MB weight buys ~19us of
PE work (~0.14 MB/us need-rate vs the queue's ~0.2+ MB/us delivery); the
main loop prefetches stage-1 of block b+2 before stage-2/attention of
block b, giving the tail weights (vw2, mqk) ~40us of runway. Engine split:
PE matmuls; ACT gelus + q'' bias-adds (Identity, co-resident with Gelu in
the table set -- no swap); DVE score masking + LN stats/rsqrt/scale;
Pool/SWDGE small-constant DMA + early-block y stores.
"""

import os
import numpy as np

import concourse.bass as bass
import concourse.tile as tile
import concourse.mybir as mybir
from concourse import bacc
from concourse.bass_utils import run_bass_kernel_spmd

AF = mybir.ActivationFunctionType
ALU = mybir.AluOpType

# problem constants
B, T, D = 4, 4096, 768
SPAN = 256
LN_EPS = 1e-5
SCALE = float(np.sqrt(D * SPAN))

P = 128
NCH = D // P          # 6 contraction chunks
N_CORES = 8
T_OWN = T // 2        # rows owned per core (2048)
T_LOC = T_OWN + SPAN  # rows incl. halo (2304)
TQ = T_OWN // P       # 16 query tiles
RING = 5
N_WARM = 16

# compute dtype: "f16" | "f32r" | "bf16" | "f32"
CDT_NAME = os.environ.get("TRN_KERNEL_CDT", "f16")
_DT = {
    "f16": (mybir.dt.float16, np.float16),
    "bf16": (mybir.dt.bfloat16, None),  # ml_dtypes.bfloat16 resolved lazily
    "f32r": (mybir.dt.float32r, np.float32),
    "f32": (mybir.dt.float32, np.float32),
}
TPB = 4               # max tiles per block (512 moving columns)
BC = TPB * P
# (start_tile, n_tiles): tapered tail so the final LN/store chain after the
# last matmul is short
BLOCKS = [(0, 4), (4, 4), (8, 4), (12, 3), (15, 1)]
assert sum(n for _, n in BLOCKS) == TQ
F32 = mybir.dt.float32
I32 = mybir.dt.int32
F16 = mybir.dt.float16


def _np_cdt():
    if CDT_NAME == "bf16":
        import ml_dtypes
        return ml_dtypes.bfloat16
    return _DT[CDT_NAME][1]


def build_module(apply_ln: bool, c_alpha: float):
    cdt = _DT[CDT_NAME][0]
    nc = bacc.Bacc("TRN2", target_bir_lowering=False, debug=False,
                   num_devices=N_CORES)

    xT = nc.dram_tensor("xT", [P, NCH * T_LOC], cdt, kind="ExternalInput")
    wd = {}
    # kw1 is stored third-major and qw1 half-major so each m-slice is one
    # contiguous DMA (a strided slice of [P, c, D] would shatter into 768B
    # packets and crawl); kw1's first third lands ~1us earlier than a half
    # would, starting the pipeline sooner. vw2+mqk ride as ONE merged
    # transfer (they arrive last back-to-back) keeping the critical
    # transfer count at exactly 11 = the DMA completion-semaphore pool.
    wd["kw1"] = nc.dram_tensor("kw1", [P, 3, NCH, 2 * P], cdt,
                               kind="ExternalInput")
    wd["qw1"] = nc.dram_tensor("qw1", [P, 2, NCH, 3 * P], cdt,
                               kind="ExternalInput")
    wd["s2w"] = nc.dram_tensor("s2w", [P, 2, NCH, D], cdt,
                               kind="ExternalInput")
    # vw1 is m-major so stage1-v's m-loop walks contiguous memory
    vw1d = nc.dram_tensor("vw1", [P, NCH, NCH, P], cdt, kind="ExternalInput")
    ud = nc.dram_tensor("u", [P, NCH], cdt, kind="ExternalInput")
    b1d = nc.dram_tensor("b1", [P, 3 * NCH], F32, kind="ExternalInput")
    b2d = nc.dram_tensor("b2", [P, NCH], F32, kind="ExternalInput")
    vb2d = nc.dram_tensor("vb2bc", [P, D], F32, kind="ExternalInput")
    maskd = nc.dram_tensor("masks", [P, 2, 3 * P], F32, kind="ExternalInput")
    if apply_ln:
        lnwd = nc.dram_tensor("lnw", [P, D], F32, kind="ExternalInput")
        lnbd = nc.dram_tensor("lnb", [P, D], F32, kind="ExternalInput")
    yd = nc.dram_tensor("y", [P, TQ, D], F16, kind="ExternalOutput")

    with tile.TileContext(nc) as tc:
        with (
            tc.tile_pool(name="work", bufs=1) as wp,
            tc.tile_pool(name="psum", bufs=1, space="PSUM") as pp,
        ):
            cp = wp  # single SBUF pool: one fewer pool-exit barrier round
            # ---- small constants (no DMA) ----
            onest = cp.tile([P, P], cdt, tag="onest", name="onest")
            nc.vector.memset(onest, 1.0)

            # ---- PE warm-up input: dummy matmuls release the HAM clock gate
            # while the first weight DMAs are still in flight ----
            wmt = wp.tile([P, 256], cdt, tag="warm", bufs=1, name="warm")
            nc.vector.memset(wmt, 0.0)

            # Warm-up train: keeps the PE busy from ~7.6us (end of framework
            # preamble) until the first weights land (~11-13us), so HAM
            # un-throttles at ~11us and the PE never re-throttles.
            for _ in range(N_WARM):
                wps = pp.tile([P, 512], F32, tag="psA", bufs=2, name="psA")
                nc.tensor.matmul(wps[:, :256], wmt[:, :P], wmt[:, :256],
                                 start=True, stop=True)

            # ---- input DMA ----
            # SWDGE (gpsimd) carries the small constants; b1 first (needed by
            # the first gelu). ALL big tensors ride the single SP/sync HWDGE
            # queue in strict first-need order, so a single ~0.36 GB/us HBM
            # stream delivers each tensor exactly when the pipeline reaches
            # it. (Two parallel queues would halve each stream's rate and
            # delay the FIRST weights, which gate the pipeline start.)
            b1t = cp.tile([P, 3 * NCH], F32, tag="b1t", name="b1t")
            nc.gpsimd.dma_start(out=b1t, in_=b1d[:])

            xp = wp.tile([P, NCH, SPAN], cdt, tag="xpp", bufs=1, name="xpp")
            nc.sync.dma_start(out=xp[:].rearrange("p c n -> p (c n)"),
                              in_=xT[:, 0:NCH * SPAN])
            wsb = {}
            wsb["kw1"] = cp.tile([P, 3, NCH, 2 * P], cdt, tag="w_kw1",
                                 name="w_kw1")
            wsb["qw1"] = cp.tile([P, 2, NCH, 3 * P], cdt, tag="w_qw1",
                                 name="w_qw1")
            s2wt = cp.tile([P, 2, NCH, D], cdt, tag="w_s2w", name="w_s2w")
            wsb["vw2"] = s2wt[:, 0]
            wsb["mqk"] = s2wt[:, 1]
            vw1t = cp.tile([P, NCH, NCH, P], cdt, tag="w_vw1", name="w_vw1")

            def xb_tile():
                return wp.tile([P, NCH, BC], cdt, tag="xT", bufs=3,
                               name="xTb")

            def xb_dma(xb, s0, ncols):
                off = NCH * (SPAN + s0 * P)
                if ncols == BC:
                    dst = xb[:].rearrange("p c n -> p (c n)")
                else:
                    dst = xb[:, :, :ncols]
                nc.sync.dma_start(out=dst, in_=xT[:, off:off + NCH * ncols])

            # ALL big tensors ride the single SP/sync queue in strict
            # first-need order (measured: one queue sustains ~0.2 MB/us in
            # the head; a second concurrent queue just splits that rate and
            # delays the first weights, which gate the pipeline start).
            # kw1/qw1 are halved so the first half lands ~3us earlier. The
            # compute prologue is WEIGHT-major (all kw1 work for halo+b0+b1,
            # then all qw1 work, then vw1) so each 1.2MB weight buys ~19us
            # of PE work -- need rate ~0.14 MB/us < DMA's ~0.2.
            def w_half_dma(nm, hh):
                nc.sync.dma_start(
                    out=wsb[nm][:, hh].rearrange("p c n -> p (c n)"),
                    in_=wd[nm][:, hh].rearrange("p c n -> p (c n)"))

            for tt in (0, 1, 2):
                nc.sync.dma_start(
                    out=wsb["kw1"][:, tt].rearrange("p c n -> p (c n)"),
                    in_=wd["kw1"][:, tt].rearrange("p c n -> p (c n)"))
            xb0 = xb_tile()
            xb_dma(xb0, 0, BC)
            xb1 = xb_tile()
            xb_dma(xb1, BLOCKS[1][0], BLOCKS[1][1] * P)
            w_half_dma("qw1", 0)
            w_half_dma("qw1", 1)
            for mlo in (0, 3):
                nc.sync.dma_start(
                    out=vw1t[:, mlo:mlo + 3].rearrange("p a c n -> p (a c n)"),
                    in_=vw1d[:, mlo:mlo + 3].rearrange("p a c n -> p (a c n)"))
            nc.sync.dma_start(
                out=s2wt[:].rearrange("p a c n -> p (a c n)"),
                in_=wd["s2w"][:].rearrange("p a c n -> p (a c n)"))

            # Small constants ride the TAIL of the same sync issue list: the
            # 11 critical transfers above fit exactly in the ~11-deep DMA
            # completion-semaphore pool, so none of them ever blocks at
            # issue; these four block briefly (needed only at ~50us+).
            vb2t = cp.tile([P, D], F32, tag="vb2t", name="vb2t")
            nc.sync.dma_start(out=vb2t, in_=vb2d[:])
            maskt = cp.tile([P, 2, 3 * P], F32, tag="maskt", name="maskt")
            nc.sync.dma_start(out=maskt, in_=maskd[:])
            ut = cp.tile([P, NCH], cdt, tag="ut", name="ut")
            nc.sync.dma_start(out=ut, in_=ud[:])
            b2t = cp.tile([P, NCH], F32, tag="b2t", name="b2t")
            nc.sync.dma_start(out=b2t, in_=b2d[:])
            if apply_ln:
                lnwt = cp.tile([P, D], F32, tag="lnwt", name="lnwt")
                nc.gpsimd.dma_start(out=lnwt, in_=lnwd[:])
                lnbt = cp.tile([P, D], F32, tag="lnbt", name="lnbt")
                nc.gpsimd.dma_start(out=lnbt, in_=lnbd[:])

            def ps512(ncols):
                t = pp.tile([P, 512], F32, tag="psA", bufs=2, name="psA")
                return t[:, :ncols]

            def ps768():
                return pp.tile([P, D], F32, tag="psB", bufs=3, name="psB")

            def w1_lhsT(w1, m, c):
                if w1 == "vw1":
                    return vw1t[:, m, c, :]
                if w1 == "kw1":
                    return wsb[w1][:, m // 2, c,
                                   (m % 2) * P:(m % 2 + 1) * P]
                return wsb[w1][:, m // 3, c, (m % 3) * P:(m % 3 + 1) * P]

            def stage1(xblk, w1, bj, ncols, out_tile=None, tag=None):
                """h = gelu(w1.T @ xT + b1) -> [P, NCH, ncols] (cdt)."""
                h = out_tile
                if h is None:
                    h = wp.tile([P, NCH, BC], cdt, tag=tag, bufs=3, name=tag)
                for m in range(NCH):
                    ps = ps512(ncols)
                    for c in range(NCH):
                        nc.tensor.matmul(
                            ps, w1_lhsT(w1, m, c),
                            xblk[:, c, :ncols],
                            start=(c == 0), stop=(c == NCH - 1))
                    nc.scalar.activation(h[:, m, :ncols], ps, AF.Gelu,
                                         bias=b1t[:, bj * NCH + m:bj * NCH + m + 1],
                                         scale=1.0)
                return h

            def stage2_q(h, qT, ncols):
                """q'' = Mqk.T @ h + b''; also alpha = u.T @ h + c.
                The alpha GEMV runs FIRST so its DVE consumer (asb add)
                drains during the six o-group matmuls -- otherwise the
                second scores group stalls on the alpha psA slot."""
                aps = pp.tile([P, 512], F32, tag="psA", bufs=2, name="psA")
                for m in range(NCH):
                    nc.tensor.matmul(
                        aps[0:1, :ncols], ut[:, m:m + 1], h[:, m, :ncols],
                        start=(m == 0), stop=(m == NCH - 1))
                asb = wp.tile([P, BC], cdt, tag="alph", bufs=2, name="alph")
                nc.vector.tensor_scalar_add(
                    asb[0:1, :ncols], aps[0:1, :ncols], c_alpha)
                for o in range(NCH):
                    ps = ps512(ncols)
                    for m in range(NCH):
                        nc.tensor.matmul(
                            ps, wsb["mqk"][:, m, o * P:(o + 1) * P],
                            h[:, m, :ncols],
                            start=(m == 0), stop=(m == NCH - 1))
                    # Identity is co-resident with Gelu in the ACT table
                    # set, so this runs on ACT with no table swap -- keeps
                    # score-prep off DVE's LN-burst FIFO
                    nc.scalar.activation(qT[:, o, :ncols], ps, AF.Identity,
                                         bias=b2t[:, o:o + 1], scale=1.0)
                return asb

            def stage2_v(h, vslot, t0, ntiles):
                """v row-tiles [rows, D] = h.T @ vw2_c + vb2_c (row-centered
                by the host-side weight fold)."""
                for t in range(ntiles):
                    ps = ps768()
                    for c0, cw in ((0, 512), (512, 256)):
                        for m in range(NCH):
                            nc.tensor.matmul(
                                ps[:, c0:c0 + cw],
                                h[:, m, t * P:(t + 1) * P],
                                wsb["vw2"][:, m, c0:c0 + cw],
                                start=(m == 0), stop=(m == NCH - 1))
                    nc.vector.tensor_add(vslot[:, t0 + t, :], ps, vb2t)

            def new_kv_slot():
                k = wp.tile([P, NCH, BC], cdt, tag="kring", bufs=RING,
                            name="kring")
                v = wp.tile([P, TPB, D], cdt, tag="vring", bufs=RING,
                            name="vring")
                return k, v

            # k/v tiles tracked by absolute tile index: kt -> (tile, pos)
            ktile = {}
            vtile = {}
            kvslot = {}

            def stage1_block(b):
                """stage-1 (q,k,v) for block b; returns (hq, hv)."""
                s0, n = BLOCKS[b]
                ncols = n * P
                if b == 0:
                    xb = xb0
                elif b == 1:
                    xb = xb1
                else:
                    xb = xb_tile()
                    xb_dma(xb, s0, ncols)
                hq = stage1(xb, "qw1", 0, ncols, tag="hq")
                kb, vb = new_kv_slot()
                kvslot[b] = (kb, vb)
                for t in range(n):
                    ktile[s0 + 2 + t] = (kb, t)
                    vtile[s0 + 2 + t] = (vb, t)
                stage1(xb, "kw1", 1, ncols, out_tile=kb)
                hv = stage1(xb, "vw1", 2, ncols, tag="hv")
                return hq, hv

            # ---- prologue: weight-major stage-1 over halo + blocks 0,1
            # (k stage-2 eliminated: the scores use g2 directly). All kw1
            # work first, then all qw1, then all vw1 -- matching the
            # single-queue DMA arrival order so the PE never waits long.
            kp, vp = new_kv_slot()
            ktile[0] = (kp, 0)
            ktile[1] = (kp, 1)
            vtile[0] = (vp, 0)
            vtile[1] = (vp, 1)
            s1, n1 = BLOCKS[1]
            kb0, vb0 = new_kv_slot()
            kvslot[0] = (kb0, vb0)
            for t in range(BLOCKS[0][1]):
                ktile[2 + t] = (kb0, t)
                vtile[2 + t] = (vb0, t)
            kb1, vb1 = new_kv_slot()
            kvslot[1] = (kb1, vb1)
            for t in range(n1):
                ktile[s1 + 2 + t] = (kb1, t)
                vtile[s1 + 2 + t] = (vb1, t)
            st1 = {}
            stage1(xp, "kw1", 1, SPAN, out_tile=kp)
            stage1(xb0, "kw1", 1, BC, out_tile=kb0)
            stage1(xb1, "kw1", 1, n1 * P, out_tile=kb1)
            hq0 = stage1(xb0, "qw1", 0, BC, tag="hq")
            hq1 = stage1(xb1, "qw1", 0, n1 * P, tag="hq")
            hv_halo = stage1(xp, "vw1", 2, SPAN, tag="hv")
            hv0 = stage1(xb0, "vw1", 2, BC, tag="hv")
            hv1 = stage1(xb1, "vw1", 2, n1 * P, tag="hv")
            st1[0] = (hq0, hv0)
            st1[1] = (hq1, hv1)
            # halo v stage-2 (first vw2 use; vw2 has landed long before)
            stage2_v(hv_halo, vp, 0, 2)

            def stage2_block(bb):
                hq, hv = st1.pop(bb)
                stage2_v(hv, kvslot[bb][1], 0, BLOCKS[bb][1])
                qT = wp.tile([P, NCH, BC], cdt, tag="qT", bufs=2, name="qT")
                asb = stage2_q(hq, qT, BLOCKS[bb][1] * P)
                return qT, asb

            s2 = {}

            # ---- main loop over blocks: prefetch stage-1 of block b+2,
            # then stage-2 + attention of block b. The FINAL block's stage-2
            # is hoisted before the previous block's attention so its
            # score-prep (DVE alpha, ACT qT adds) is enqueued ahead of the
            # taper's LN bursts and never stalls the PE ----
            for b, (s0, n) in enumerate(BLOCKS):
                ncols = n * P
                if b + 2 < len(BLOCKS):
                    st1[b + 2] = stage1_block(b + 2)
                s2[b] = stage2_block(b)
                qT, asb = s2.pop(b)

                # ---- attention: scoresT per k-tile over the whole q block.
                # Groups are emitted WIDEST-W first: the wide groups'
                # ~160ns/chunk cadence hides the final qT o-group add
                # (ACT, ~690ns) that the c=5 chunks depend on; narrow
                # W=128 groups (56ns/chunk) cannot. ----
                scs = {}
                dlts = sorted(range(n + 2),
                              key=lambda d: -(min(n - 1, d) - max(0, d - 2)))
                for dlt in dlts:
                    p_lo = max(0, dlt - 2)
                    p_hi = min(n - 1, dlt)
                    W = (p_hi - p_lo + 1) * P
                    qoff = p_lo * P
                    moff = (2 - (dlt - p_lo)) * P
                    msel = 1 if (b == 0 and dlt < 2) else 0
                    kts, kpos = ktile[s0 + dlt]
                    ps = ps512(W)
                    for c in range(NCH):
                        nc.tensor.matmul(
                            ps, kts[:, c, kpos * P:(kpos + 1) * P],
                            qT[:, c, qoff:qoff + W],
                            start=(c == 0), stop=False)
                    # + alpha_i broadcast over keys (rank-1, K=1)
                    nc.tensor.matmul(
                        ps, onest[0:1, :P], asb[0:1, qoff:qoff + W],
                        start=False, stop=True)
                    sc = wp.tile([P, 3 * P], cdt, tag="sc", bufs=8, name="sc")
                    nc.vector.tensor_mul(sc[:, :W], ps,
                                         maskt[:, msel, moff:moff + W])
                    scs[dlt] = (sc, qoff)

                # y = scoresT.T @ v accumulated over the 3 band tiles,
                # chunk-major so bn_stats on cols [0:512] overlaps the
                # [512:768] matmuls. v rows are pre-centered, so y is
                # zero-mean and LN is y * rsqrt(var+eps): rsqrt via DVE
                # magic-seed + 1 Newton step (batched over tile pairs),
                # scale split DVE (cols 0:384) / ACT Copy (cols 384:768).
                yb = wp.tile([P, TPB, D], F16, tag="yb", bufs=3, name="yb")
                mvb = wp.tile([P, TPB, 2], F32, tag="mvb", bufs=2, name="mvb")
                for p in range(n):
                    if b == len(BLOCKS) - 1 and not apply_ln:
                        # final block's single y tile accumulates in TWO psA
                        # slots (512+256) instead of psB, so it never waits
                        # on block 3's LN chains to free a psB slot
                        psy0 = ps512(512)
                        psy1 = ps512(256)
                        chunks = ((psy0, 0, 512), (psy1, 512, 256))
                    else:
                        psy = ps768()
                        chunks = ((psy, 0, 512), (psy, 512, 256))
                    st = wp.tile([P, 2, 6], F32, tag="st", bufs=4, name="st")
                    for si, (pst, c0v, cw) in enumerate(chunks):
                        dst = pst[:, 0:cw] if pst is not chunks[0][0] \
                            else pst[:, c0v:c0v + cw]
                        for j, dlt in enumerate((p, p + 1, p + 2)):
                            sc, qoff = scs[dlt]
                            soff = p * P - qoff
                            vts, vpos = vtile[s0 + dlt]
                            nc.tensor.matmul(
                                dst,
                                sc[:, soff:soff + P],
                                vts[:, vpos, c0v:c0v + cw],
                                start=(j == 0), stop=(j == 2))
                        # chunk-aligned stats groups: group 0 (512 cols)
                        # fully overlaps the second chunk's MMs
                        nc.vector.bn_stats(st[:, si, :], dst)
                    nc.vector.bn_aggr(mvb[:, p, :], st)
                    # rr = rsqrt(var + eps)
                    vh = wp.tile([P, 1], F32, tag="vh", bufs=2, name="vh")
                    rr = wp.tile([P, 1], F32, tag="rr", bufs=2, name="rr")
                    ta = wp.tile([P, 1], F32, tag="ta", bufs=2, name="ta")
                    v2 = wp.tile([P, 1], F32, tag="v2", bufs=2, name="v2")
                    nc.vector.tensor_scalar_add(v2, mvb[:, p, 1:2], LN_EPS)
                    nc.vector.tensor_scalar_mul(vh, v2, -0.5)
                    nc.vector.tensor_copy(ta, v2[:].bitcast(I32))
                    nc.vector.tensor_scalar(
                        out=ta, in0=ta, scalar1=-0.5,
                        scalar2=float(0x5F375A86), op0=ALU.mult, op1=ALU.add)
                    nc.vector.tensor_copy(rr[:].bitcast(I32), ta)
                    nc.vector.tensor_mul(ta, rr, rr)
                    nc.vector.tensor_scalar(
                        out=ta, in0=ta, scalar1=vh[:, 0:1], scalar2=1.5,
                        op0=ALU.mult, op1=ALU.add)
                    nc.vector.tensor_mul(rr, rr, ta)
                    if apply_ln:
                        yf = wp.tile([P, D], F32, tag="yf", bufs=2, name="yf")
                        nc.vector.tensor_scalar(
                            out=yf, in0=psy,
                            scalar1=mvb[:, p, 0:1], scalar2=rr[:, 0:1],
                            op0=ALU.subtract, op1=ALU.mult)
                        nc.vector.tensor_mul(yf, yf, lnwt)
                        nc.vector.tensor_add(yb[:, p, :], yf, lnbt)
                    elif b == len(BLOCKS) - 1:
                        # final block: no gelus follow, so the ACT engine is
                        # free -- split the scale DVE/ACT and the store in
                        # halves to shorten the post-last-matmul chain
                        nc.vector.tensor_scalar_mul(yb[:, p, 0:384],
                                                    psy0[:, 0:384],
                                                    rr[:, 0:1])
                        nc.scalar.activation(
                            yb[:, p, 384:512], psy0[:, 384:512], AF.Copy,
                            bias=0.0, scale=rr[:, 0:1])
                        nc.scalar.activation(
                            yb[:, p, 512:768], psy1[:, 0:256], AF.Copy,
                            bias=0.0, scale=rr[:, 0:1])
                        # NOTE: issuing this store on gpsimd instead of sync
                        # triggers a global ~20ns/MM slowdown (+46us!) --
                        # keep it on the sync ring
                        nc.sync.dma_start(out=yd[:, s0 + p, :],
                                          in_=yb[:, p, :])
                    else:
                        # v rows are host-centered, so y is zero-mean:
                        # normalize is a pure per-row scale by rr
                        nc.vector.tensor_scalar_mul(yb[:, p, :], psy,
                                                    rr[:, 0:1])
                    if b < len(BLOCKS) - 1:
                        yeng = nc.sync if b >= len(BLOCKS) - 2 else nc.gpsimd
                        yeng.dma_start(out=yd[:, s0 + p, :], in_=yb[:, p, :])

    nc.compile()
    return nc


def _make_masks(h):
    jj, ii = np.mgrid[0:P, 0:P]
    diag = (ii >= jj).astype(np.float32)
    full = np.ones((P, P), np.float32)
    strict = (ii < jj).astype(np.float32)
    gen = np.concatenate([diag, full, strict], axis=1) / SCALE
    if h == 0:
        z = np.zeros((P, P), np.float32)
        blk0 = np.concatenate([diag / SCALE, z, z], axis=1)
    else:
        blk0 = gen
    return np.stack([gen, blk0], axis=1)  # [P, 2, 384]


def kernel(**inputs):
    x = np.asarray(inputs["x"], np.float32)
    npdt = _np_cdt()

    ln_w = np.asarray(inputs["ln_w"], np.float32)
    ln_b = np.asarray(inputs["ln_b"], np.float32)
    apply_ln = not (np.all(ln_w == 1.0) and np.all(ln_b == 0.0))

    qw2 = np.asarray(inputs["qw2"], np.float32)
    kw2 = np.asarray(inputs["kw2"], np.float32)
    qb2 = np.asarray(inputs["qb2"], np.float32)
    kb2 = np.asarray(inputs["kb2"], np.float32)
    mqk = qw2 @ kw2.T                      # [768, 768]
    bpp = qb2 @ kw2.T                      # [768]
    u = qw2 @ kb2                          # [768]
    c_alpha = float(qb2 @ kb2)

    nc = build_module(apply_ln, c_alpha)

    def warr(w):
        w = np.asarray(w, np.float32)
        return np.ascontiguousarray(
            w.reshape(NCH, P, D).transpose(1, 0, 2)).astype(npdt)

    def warr_halves(w):
        # [P, c, D] -> [P, half, c, 3*P] (m-halves contiguous for DMA)
        a = warr(w).reshape(P, NCH, 2, 3 * P)
        return np.ascontiguousarray(a.transpose(0, 2, 1, 3))

    def warr_thirds(w):
        # [P, c, D] -> [P, third, c, 2*P] (m-thirds contiguous for DMA)
        a = warr(w).reshape(P, NCH, 3, 2 * P)
        return np.ascontiguousarray(a.transpose(0, 2, 1, 3))

    # center v rows via the weights: vw2_c rows sum to zero, vb2_c mean 0
    vw2 = np.asarray(inputs["vw2"], np.float32)
    vw2_c = vw2 - vw2.mean(axis=1, keepdims=True)
    vb2 = np.asarray(inputs["vb2"], np.float32)
    vb2_c = vb2 - vb2.mean()

    # vw1 m-major: vw1m[p, m, c, col] = vw1[c*P+p, m*P+col]
    vw1m = np.ascontiguousarray(
        np.asarray(inputs["vw1"], np.float32)
        .reshape(NCH, P, NCH, P).transpose(1, 2, 0, 3)).astype(npdt)
    wmats = {
        "qw1": warr_halves(inputs["qw1"]), "kw1": warr_thirds(inputs["kw1"]),
        "vw1": vw1m,
        "s2w": np.ascontiguousarray(
            np.stack([warr(vw2_c), warr(mqk)], axis=1)),
    }
    b1 = np.ascontiguousarray(
        np.stack([inputs["qb1"], inputs["kb1"], inputs["vb1"]])
        .astype(np.float32).reshape(3, NCH, P).transpose(2, 0, 1)
        .reshape(P, 3 * NCH))
    b2 = np.ascontiguousarray(bpp.reshape(NCH, P).transpose(1, 0))
    u_arr = np.ascontiguousarray(
        u.reshape(NCH, P).transpose(1, 0)).astype(npdt)
    vb2bc = np.ascontiguousarray(np.broadcast_to(vb2_c, (P, D)))

    in_maps = []
    for core in range(N_CORES):
        bi, h = core // 2, core % 2
        xl = np.zeros((T_LOC, D), np.float32)
        lo = h * T_OWN - SPAN
        if h == 0:
            xl[SPAN:] = x[bi, 0:T_OWN]
        else:
            xl[:] = x[bi, lo:lo + T_LOC]
        xt3 = xl.T.reshape(NCH, P, T_LOC).transpose(1, 0, 2)  # [P, c, r]
        segs = [xt3[:, :, 0:SPAN].reshape(P, -1)]
        for (s0, n) in BLOCKS:
            c0 = SPAN + s0 * P
            segs.append(xt3[:, :, c0:c0 + n * P].reshape(P, -1))
        xTn = np.ascontiguousarray(np.concatenate(segs, axis=1)).astype(npdt)
        m = {
            "xT": xTn, "b1": b1, "b2": b2, "u": u_arr, "vb2bc": vb2bc,
            "masks": np.ascontiguousarray(_make_masks(h)),
        }
        m.update(wmats)
        if apply_ln:
            m["lnw"] = np.ascontiguousarray(np.broadcast_to(ln_w, (P, D)))
            m["lnb"] = np.ascontiguousarray(np.broadcast_to(ln_b, (P, D)))
        in_maps.append(m)

    trace = os.environ.get("TRN_KERNEL_TRACE", "0") == "1"
    res = run_bass_kernel_spmd(nc, in_maps, core_ids=list(range(N_CORES)),
                               trace=trace)
    if trace and res.exec_time_ns is not None:
        print(f"HW exec time: {res.exec_time_ns} ns")
        print(f"mean exec time: {res.mean_exec_time_ns} ns")
        if res.instructions_and_trace is not None:
            print(f"trace: {res.instructions_and_trace[1]}")

    out = np.empty((B, T, D), np.float32)
    for core in range(N_CORES):
        bi, h = core // 2, core % 2
        yc = np.asarray(res.results[core]["y"], np.float32)  # [P, TQ, D]
        out[bi, h * T_OWN:(h + 1) * T_OWN] = (
            yc.transpose(1, 0, 2).reshape(T_OWN, D))
    return out
